# revision 7
# baseline (speedup 1.0000x reference)
"""AttnDecoderRNN single-step decoder on 8 Trainium2 NeuronCores.

Sharding:
  - Front (embedding gather, Bahdanau attention, combine+relu, GRU cell):
    data-parallel over batch (32 rows/core).
  - h_new all-gathered (bf16) across the 8 cores.
  - Final fc + log_softmax: tensor-parallel over the vocab dimension
    (6283 columns/core); log-softmax denominators all-gathered and the
    normalization applied locally.

Weights are pre-packed on the host at staging time (transposed to
[in, out] layout and cast to bf16) — a one-time model-load transform.
Activations (hidden, encoder_outputs, tokens) are staged untouched.
"""
import sys

if "/opt/trn_rl_repo" not in sys.path:
    sys.path.insert(0, "/opt/trn_rl_repo")

import numpy as np
import ml_dtypes

import concourse.bass as bass
import concourse.tile as tile
from concourse import bacc, mybir
from concourse import bass_utils
from concourse.masks import make_identity

BF16 = mybir.dt.bfloat16
F32 = mybir.dt.float32
I32 = mybir.dt.int32
AF = mybir.ActivationFunctionType

H, V, B, L = 512, 50257, 256, 50
NC = 8
BS = B // NC            # 32 batch rows per core
VS = (V + NC - 1) // NC  # 6283 vocab columns per core
VPAD = VS * NC           # 50264
KT = H // 128            # 4 contraction tiles of 128
RG = [list(range(NC))]

# vocab column tiles (PSUM bank limit: 512 f32 per matmul)
N_TILES = []
_off = 0
while _off < VS:
    n = min(512, VS - _off)
    N_TILES.append((_off, n))
    _off += n

_CACHE = {}


def _build():
    nc = bacc.Bacc("TRN2", target_bir_lowering=False, debug=False, num_devices=NC)

    # ---- I/O ----
    tok = nc.dram_tensor("tok", [BS, 1], I32, kind="ExternalInput")
    h0 = nc.dram_tensor("h0", [BS, H], F32, kind="ExternalInput")
    enc = nc.dram_tensor("enc", [L, BS, H], F32, kind="ExternalInput")
    embt = nc.dram_tensor("embt", [V, H], BF16, kind="ExternalInput")
    attn_wT = nc.dram_tensor("attn_wT", [2 * H, L], BF16, kind="ExternalInput")
    attn_b = nc.dram_tensor("attn_b", [1, L], BF16, kind="ExternalInput")
    comb_wT = nc.dram_tensor("comb_wT", [2 * H, H], BF16, kind="ExternalInput")
    comb_b = nc.dram_tensor("comb_b", [1, H], BF16, kind="ExternalInput")
    w_ihT = nc.dram_tensor("w_ihT", [H, 3 * H], BF16, kind="ExternalInput")
    w_hhT = nc.dram_tensor("w_hhT", [H, 3 * H], BF16, kind="ExternalInput")
    b_ih = nc.dram_tensor("b_ih", [1, 3 * H], BF16, kind="ExternalInput")
    b_hh = nc.dram_tensor("b_hh", [1, 3 * H], BF16, kind="ExternalInput")
    fc_wT = nc.dram_tensor("fc_wT", [H, VS], BF16, kind="ExternalInput")
    fc_b = nc.dram_tensor("fc_b", [1, VS], BF16, kind="ExternalInput")

    out_logp = nc.dram_tensor("out_logp", [B, VS], F32, kind="ExternalOutput")
    out_h = nc.dram_tensor("out_h", [BS, H], F32, kind="ExternalOutput")
    out_attn = nc.dram_tensor("out_attn", [BS, L], F32, kind="ExternalOutput")

    cc_h_in = nc.dram_tensor("cc_h_in", [BS, H], BF16)
    cc_h_out = nc.dram_tensor("cc_h_out", [B, H], BF16, addr_space="Shared")
    cc_s_in = nc.dram_tensor("cc_s_in", [1, B], F32)
    cc_s_out = nc.dram_tensor("cc_s_out", [NC, B], F32, addr_space="Shared")

    with tile.TileContext(nc) as tc:
        with (
            tc.tile_pool(name="singles", bufs=1) as sg,
            tc.tile_pool(name="work", bufs=2) as wk,
            tc.tile_pool(name="encp", bufs=8) as encp,
            tc.tile_pool(name="outp", bufs=4) as outp,
            tc.tile_pool(name="trp", bufs=2, space="PSUM") as trp,
            tc.tile_pool(name="gp", bufs=3, space="PSUM") as gp,
            tc.tile_pool(name="zp", bufs=3, space="PSUM") as zp,
        ):
            # ---- constants ----
            id_bf = sg.tile([128, 128], BF16, tag="id_bf")
            make_identity(nc, id_bf[:])
            id_f = sg.tile([128, 128], F32, tag="id_f")
            make_identity(nc, id_f[:])
            ones_bf = sg.tile([1, 128], BF16, tag="ones")
            nc.vector.memset(ones_bf[:], 1.0)

            # ---- embedding gather ----
            tok_sb = sg.tile([BS, 1], I32, tag="tok")
            nc.sync.dma_start(out=tok_sb[:], in_=tok.ap())
            emb_own = sg.tile([BS, H], BF16, tag="embrow")
            nc.gpsimd.indirect_dma_start(
                out=emb_own[:], out_offset=None, in_=embt.ap(),
                in_offset=bass.IndirectOffsetOnAxis(ap=tok_sb[:, 0:1], axis=0),
            )
            h0_sb = sg.tile([BS, H], F32, tag="h0")
            nc.sync.dma_start(out=h0_sb[:], in_=h0.ap())

            # ---- feature-major transposes of embedded and h0 ----
            embT = []
            h0T = []
            for k in range(KT):
                pe = trp.tile([128, BS], BF16, tag="tr")
                nc.tensor.transpose(out=pe[:], in_=emb_own[:, 128 * k:128 * (k + 1)],
                                    identity=id_bf[:BS, :BS])
                t = sg.tile([128, BS], BF16, tag=f"embT{k}")
                nc.vector.tensor_copy(out=t[:], in_=pe[:])
                embT.append(t)

                pf = trp.tile([128, BS], F32, tag="tr")
                nc.tensor.transpose(out=pf[:], in_=h0_sb[:, 128 * k:128 * (k + 1)],
                                    identity=id_f[:BS, :BS])
                t2 = sg.tile([128, BS], BF16, tag=f"h0T{k}")
                nc.vector.tensor_copy(out=t2[:], in_=pf[:])
                h0T.append(t2)

            # ---- attention scores + softmax ----
            aw_sb = sg.tile([128, 2 * KT, L], BF16, tag="aw")
            nc.sync.dma_start(out=aw_sb[:],
                              in_=attn_wT.ap().rearrange("(k p) l -> p k l", p=128))
            ab_sb = sg.tile([1, L], BF16, tag="ab")
            nc.sync.dma_start(out=ab_sb[:], in_=attn_b.ap())

            psc = gp.tile([BS, 512], F32, tag="gpsum")
            for k in range(KT):
                nc.tensor.matmul(out=psc[:, :L], lhsT=embT[k][:], rhs=aw_sb[:, k, :],
                                 start=(k == 0), stop=False)
            for k in range(KT):
                nc.tensor.matmul(out=psc[:, :L], lhsT=h0T[k][:], rhs=aw_sb[:, KT + k, :],
                                 start=False, stop=False)
            nc.tensor.matmul(out=psc[:, :L], lhsT=ones_bf[0:1, :BS], rhs=ab_sb[:],
                             start=False, stop=True)

            # scores are tiny (|s| < ~1), exp is safe without max subtraction
            e_sb = sg.tile([BS, L], F32, tag="esb")
            ssum = sg.tile([BS, 1], F32, tag="ssum")
            nc.scalar.activation(out=e_sb[:], in_=psc[:, :L], func=AF.Exp,
                                 accum_out=ssum[:])
            rinv = sg.tile([BS, 1], F32, tag="rinv")
            nc.vector.reciprocal(rinv[:], ssum[:])
            attnw = sg.tile([BS, L], F32, tag="attnw")
            nc.vector.tensor_scalar_mul(attnw[:], e_sb[:], rinv[:, 0:1])
            nc.sync.dma_start(out=out_attn.ap(), in_=attnw[:])

            # ---- attn_applied via masked accumulating matmuls ----
            pwt = trp.tile([L, BS], F32, tag="tr")
            nc.tensor.transpose(out=pwt[:], in_=attnw[:], identity=id_f[:BS, :BS])
            wt_bf = sg.tile([L, BS], BF16, tag="wtbf")
            nc.vector.tensor_copy(out=wt_bf[:], in_=pwt[:])
            wmask = sg.tile([L, BS, BS], BF16, tag="wmask")
            nc.vector.tensor_copy(
                out=wmask[:],
                in_=wt_bf[:].rearrange("l (o j) -> l o j", o=1).to_broadcast([L, BS, BS]),
            )
            nc.gpsimd.affine_select(
                out=wmask[:], in_=wmask[:], compare_op=mybir.AluOpType.is_equal,
                fill=0.0, base=0, pattern=[[-1, BS], [1, BS]], channel_multiplier=0,
            )
            patt = gp.tile([BS, 512], F32, tag="gpsum")
            for b in range(BS):
                et = encp.tile([L, H], F32, tag="enc")
                nc.sync.dma_start(out=et[:], in_=enc.ap()[:, b, :])
                ebf = et[:].bitcast(BF16).rearrange("l (n two) -> l n two", two=2)[:, :, 1]
                nc.tensor.matmul(out=patt[:], lhsT=wmask[:, b, :], rhs=ebf,
                                 start=(b == 0), stop=(b == BS - 1))
            aa_bf = sg.tile([BS, H], BF16, tag="aabf")
            nc.vector.tensor_copy(out=aa_bf[:], in_=patt[:])

            aaT = []
            for k in range(KT):
                pe = trp.tile([128, BS], BF16, tag="tr")
                nc.tensor.transpose(out=pe[:], in_=aa_bf[:, 128 * k:128 * (k + 1)],
                                    identity=id_bf[:BS, :BS])
                t = sg.tile([128, BS], BF16, tag=f"aaT{k}")
                nc.vector.tensor_copy(out=t[:], in_=pe[:])
                aaT.append(t)

            # ---- combine + relu ----
            cw_sb = sg.tile([128, 2 * KT, H], BF16, tag="cw")
            nc.sync.dma_start(out=cw_sb[:],
                              in_=comb_wT.ap().rearrange("(k p) n -> p k n", p=128))
            cb_sb = sg.tile([1, H], BF16, tag="cb")
            nc.sync.dma_start(out=cb_sb[:], in_=comb_b.ap())
            px = gp.tile([BS, 512], F32, tag="gpsum")
            for k in range(KT):
                nc.tensor.matmul(out=px[:, :H], lhsT=embT[k][:], rhs=cw_sb[:, k, :],
                                 start=(k == 0), stop=False)
            for k in range(KT):
                nc.tensor.matmul(out=px[:, :H], lhsT=aaT[k][:], rhs=cw_sb[:, KT + k, :],
                                 start=False, stop=False)
            nc.tensor.matmul(out=px[:, :H], lhsT=ones_bf[0:1, :BS], rhs=cb_sb[:],
                             start=False, stop=True)
            x_bf = sg.tile([BS, H], BF16, tag="xbf")
            nc.scalar.activation(out=x_bf[:], in_=px[:, :H], func=AF.Relu)

            xT = []
            for k in range(KT):
                pe = trp.tile([128, BS], BF16, tag="tr")
                nc.tensor.transpose(out=pe[:], in_=x_bf[:, 128 * k:128 * (k + 1)],
                                    identity=id_bf[:BS, :BS])
                t = sg.tile([128, BS], BF16, tag=f"xT{k}")
                nc.vector.tensor_copy(out=t[:], in_=pe[:])
                xT.append(t)

            # ---- GRU cell ----
            wih_sb = sg.tile([128, KT, 3 * H], BF16, tag="wih")
            nc.sync.dma_start(out=wih_sb[:],
                              in_=w_ihT.ap().rearrange("(k p) n -> p k n", p=128))
            whh_sb = sg.tile([128, KT, 3 * H], BF16, tag="whh")
            nc.sync.dma_start(out=whh_sb[:],
                              in_=w_hhT.ap().rearrange("(k p) n -> p k n", p=128))
            bih_sb = sg.tile([1, 3 * H], BF16, tag="bih")
            nc.sync.dma_start(out=bih_sb[:], in_=b_ih.ap())
            bhh_sb = sg.tile([1, 3 * H], BF16, tag="bhh")
            nc.sync.dma_start(out=bhh_sb[:], in_=b_hh.ap())

            gi_sb = []
            for j in range(3):
                pg = gp.tile([BS, 512], F32, tag="gpsum")
                for k in range(KT):
                    nc.tensor.matmul(out=pg[:, :H], lhsT=xT[k][:],
                                     rhs=wih_sb[:, k, H * j:H * (j + 1)],
                                     start=(k == 0), stop=False)
                nc.tensor.matmul(out=pg[:, :H], lhsT=ones_bf[0:1, :BS],
                                 rhs=bih_sb[:, H * j:H * (j + 1)],
                                 start=False, stop=True)
                t = sg.tile([BS, H], F32, tag=f"gi{j}")
                nc.vector.tensor_copy(out=t[:], in_=pg[:, :H])
                gi_sb.append(t)

            r_sb = sg.tile([BS, H], F32, tag="r")
            z_gate = sg.tile([BS, H], F32, tag="zg")
            n_sb = sg.tile([BS, H], F32, tag="n")
            hnew = sg.tile([BS, H], F32, tag="hnew")
            for j in range(3):
                pg = gp.tile([BS, 512], F32, tag="gpsum")
                for k in range(KT):
                    nc.tensor.matmul(out=pg[:, :H], lhsT=h0T[k][:],
                                     rhs=whh_sb[:, k, H * j:H * (j + 1)],
                                     start=(k == 0), stop=False)
                nc.tensor.matmul(out=pg[:, :H], lhsT=ones_bf[0:1, :BS],
                                 rhs=bhh_sb[:, H * j:H * (j + 1)],
                                 start=False, stop=True)
                if j == 0:
                    pre = sg.tile([BS, H], F32, tag="pre0")
                    nc.vector.tensor_add(out=pre[:], in0=gi_sb[0][:], in1=pg[:, :H])
                    nc.scalar.activation(out=r_sb[:], in_=pre[:], func=AF.Sigmoid)
                elif j == 1:
                    pre = sg.tile([BS, H], F32, tag="pre1")
                    nc.vector.tensor_add(out=pre[:], in0=gi_sb[1][:], in1=pg[:, :H])
                    nc.scalar.activation(out=z_gate[:], in_=pre[:], func=AF.Sigmoid)
                else:
                    hnr = sg.tile([BS, H], F32, tag="hnr")
                    nc.vector.tensor_mul(out=hnr[:], in0=r_sb[:], in1=pg[:, :H])
                    pre = sg.tile([BS, H], F32, tag="pre2")
                    nc.vector.tensor_add(out=pre[:], in0=gi_sb[2][:], in1=hnr[:])
                    nc.scalar.activation(out=n_sb[:], in_=pre[:], func=AF.Tanh)

            d_sb = sg.tile([BS, H], F32, tag="d")
            nc.vector.tensor_tensor(out=d_sb[:], in0=h0_sb[:], in1=n_sb[:],
                                    op=mybir.AluOpType.subtract)
            e2_sb = sg.tile([BS, H], F32, tag="e2")
            nc.vector.tensor_mul(out=e2_sb[:], in0=z_gate[:], in1=d_sb[:])
            nc.vector.tensor_add(out=hnew[:], in0=n_sb[:], in1=e2_sb[:])
            nc.sync.dma_start(out=out_h.ap(), in_=hnew[:])

            # ---- all-gather h_new (bf16) ----
            h_bf = sg.tile([BS, H], BF16, tag="hbf")
            nc.scalar.activation(out=h_bf[:], in_=hnew[:], func=AF.Copy)
            nc.sync.dma_start(out=cc_h_in.ap(), in_=h_bf[:])
            nc.gpsimd.collective_compute(
                "AllGather", mybir.AluOpType.bypass, replica_groups=RG,
                ins=[cc_h_in.ap()], outs=[cc_h_out.ap()],
            )

            hT = [sg.tile([128, B], BF16, tag=f"hT{k}", name=f"hT{k}") for k in range(KT)]
            for bt in range(2):
                hf = wk.tile([128, H], BF16, tag="hfull")
                nc.sync.dma_start(out=hf[:], in_=cc_h_out.ap()[128 * bt:128 * (bt + 1), :])
                for k in range(KT):
                    pe = trp.tile([128, 128], BF16, tag="tr")
                    nc.tensor.transpose(out=pe[:], in_=hf[:, 128 * k:128 * (k + 1)],
                                        identity=id_bf[:])
                    nc.vector.tensor_copy(out=hT[k][:, 128 * bt:128 * (bt + 1)], in_=pe[:])

            # ---- fc matmul + exp/σ stats ----
            # fc_wT is made fully SBUF-resident via 4 big DMAs that carry no
            # dependency on the front, so they stream during front + AllGather.
            fcb_sb = sg.tile([1, VS], BF16, tag="fcb")
            nc.sync.dma_start(out=fcb_sb[:], in_=fc_b.ap())
            wz_res = sg.tile([128, KT, VS], BF16, tag="wzres")
            for k in range(KT):
                nc.sync.dma_start(out=wz_res[:, k, :],
                                  in_=fc_wT.ap()[128 * k:128 * (k + 1), :])
            z_sb = [sg.tile([128, VS], BF16, tag=f"z{bt}", name=f"z{bt}") for bt in range(2)]
            stats = [sg.tile([128, len(N_TILES)], F32, tag=f"st{bt}", name=f"stats{bt}") for bt in range(2)]

            for nt, (ncur, n) in enumerate(N_TILES):
                for bt in range(2):
                    pz = zp.tile([128, 512], F32, tag="zpsum")
                    for k in range(KT):
                        nc.tensor.matmul(out=pz[:, :n],
                                         lhsT=hT[k][:, 128 * bt:128 * (bt + 1)],
                                         rhs=wz_res[:, k, ncur:ncur + n],
                                         start=(k == 0), stop=False)
                    nc.tensor.matmul(out=pz[:, :n], lhsT=ones_bf[0:1, :],
                                     rhs=fcb_sb[:, ncur:ncur + n], start=False, stop=True)
                    nc.vector.tensor_copy(out=z_sb[bt][:, ncur:ncur + n], in_=pz[:, :n])
                    esc = wk.tile([128, 512], BF16, tag="esc")
                    nc.scalar.activation(out=esc[:, :n], in_=pz[:, :n],
                                         func=AF.Exp, accum_out=stats[bt][:, nt:nt + 1])

            # ---- all-gather softmax denominators ----
            for bt in range(2):
                s_own = sg.tile([128, 1], F32, tag=f"sown{bt}")
                nc.vector.reduce_sum(s_own[:], stats[bt][:, 0:len(N_TILES)],
                                     axis=mybir.AxisListType.X)
                pt = trp.tile([1, 128], F32, tag="tr")
                nc.tensor.transpose(out=pt[:], in_=s_own[:], identity=id_f[:])
                srow = sg.tile([1, 128], F32, tag=f"srow{bt}")
                nc.vector.tensor_copy(out=srow[:], in_=pt[:])
                nc.sync.dma_start(out=cc_s_in.ap()[:, 128 * bt:128 * (bt + 1)], in_=srow[:])
            nc.gpsimd.collective_compute(
                "AllGather", mybir.AluOpType.bypass, replica_groups=RG,
                ins=[cc_s_in.ap()], outs=[cc_s_out.ap()],
            )

            lse = []
            for bt in range(2):
                s_all = sg.tile([128, NC], F32, tag=f"sall{bt}")
                nc.sync.dma_start(
                    out=s_all[:],
                    in_=cc_s_out.ap().rearrange("r b -> b r")[128 * bt:128 * (bt + 1), :])
                s_tot = sg.tile([128, 1], F32, tag=f"stot{bt}")
                nc.vector.reduce_sum(s_tot[:], s_all[:], axis=mybir.AxisListType.X)
                ls = sg.tile([128, 1], F32, tag=f"lse{bt}")
                nc.scalar.activation(out=ls[:], in_=s_tot[:], func=AF.Ln)
                lse.append(ls)

            # ---- normalize + store ----
            for bt in range(2):
                for ncur, n in N_TILES:
                    o_t = outp.tile([128, 512], F32, tag="ost")
                    nc.vector.tensor_scalar_sub(o_t[:, :n], z_sb[bt][:, ncur:ncur + n],
                                                lse[bt][:, 0:1])
                    nc.sync.dma_start(
                        out=out_logp.ap()[128 * bt:128 * (bt + 1), ncur:ncur + n],
                        in_=o_t[:, :n])

    nc.compile()
    return nc


def _stage(inputs):
    """Build the 8 per-core in_maps from the full-size inputs."""
    bf = ml_dtypes.bfloat16
    tok = np.asarray(inputs["input_tokens"]).astype(np.int32).reshape(B, 1)
    hidden = np.ascontiguousarray(np.asarray(inputs["hidden"], np.float32))[0]  # [B,H]
    enc = np.ascontiguousarray(np.asarray(inputs["encoder_outputs"], np.float32))
    emb_bf = np.asarray(inputs["emb"], np.float32).astype(bf)
    attn_wT = np.ascontiguousarray(np.asarray(inputs["attn_w"], np.float32).T).astype(bf)
    attn_b = np.asarray(inputs["attn_b"], np.float32).reshape(1, L).astype(bf)
    comb_wT = np.ascontiguousarray(np.asarray(inputs["comb_w"], np.float32).T).astype(bf)
    comb_b = np.asarray(inputs["comb_b"], np.float32).reshape(1, H).astype(bf)
    w_ihT = np.ascontiguousarray(np.asarray(inputs["w_ih"], np.float32).T).astype(bf)
    w_hhT = np.ascontiguousarray(np.asarray(inputs["w_hh"], np.float32).T).astype(bf)
    b_ih = np.asarray(inputs["b_ih"], np.float32).reshape(1, 3 * H).astype(bf)
    b_hh = np.asarray(inputs["b_hh"], np.float32).reshape(1, 3 * H).astype(bf)

    fc_w = np.asarray(inputs["fc_w"], np.float32)
    fc_b = np.asarray(inputs["fc_b"], np.float32)
    fc_w_pad = np.zeros((VPAD, H), np.float32)
    fc_w_pad[:V] = fc_w
    fc_b_pad = np.full((VPAD,), -1e30, np.float32)
    fc_b_pad[:V] = fc_b

    in_maps = []
    for c in range(NC):
        b0 = c * BS
        v0 = c * VS
        in_maps.append({
            "tok": tok[b0:b0 + BS],
            "h0": np.ascontiguousarray(hidden[b0:b0 + BS]),
            "enc": np.ascontiguousarray(enc[:, b0:b0 + BS, :]),
            "embt": emb_bf,
            "attn_wT": attn_wT,
            "attn_b": attn_b,
            "comb_wT": comb_wT,
            "comb_b": comb_b,
            "w_ihT": w_ihT,
            "w_hhT": w_hhT,
            "b_ih": b_ih,
            "b_hh": b_hh,
            "fc_wT": np.ascontiguousarray(fc_w_pad[v0:v0 + VS].T).astype(bf),
            "fc_b": fc_b_pad[v0:v0 + VS].reshape(1, VS).astype(bf),
        })
    return in_maps


def _run(inputs, trace=False, trace_cores=None):
    if "nc" not in _CACHE:
        _CACHE["nc"] = _build()
    nc = _CACHE["nc"]
    in_maps = _stage(inputs)
    res = bass_utils.run_bass_kernel_spmd(
        nc, in_maps, core_ids=list(range(NC)), trace=trace, trace_cores=trace_cores)
    logp = np.concatenate([res.results[c]["out_logp"] for c in range(NC)], axis=1)[:, :V]
    h_new = np.concatenate([res.results[c]["out_h"] for c in range(NC)], axis=0)[None]
    attnw = np.concatenate([res.results[c]["out_attn"] for c in range(NC)], axis=0)
    return (logp, h_new, attnw), res


def kernel(**inputs):
    out, _ = _run(inputs, trace=False)
    return out


# revision 9
# speedup vs baseline: 1.0510x; 1.0510x over previous
"""AttnDecoderRNN single-step decoder on 8 Trainium2 NeuronCores.

Sharding:
  - Front (embedding gather, Bahdanau attention, combine+relu, GRU cell):
    data-parallel over batch (32 rows/core).
  - h_new all-gathered (bf16) across the 8 cores.
  - Final fc + log_softmax: tensor-parallel over the vocab dimension
    (6283 columns/core); log-softmax denominators all-gathered and the
    normalization applied locally.

Weights are pre-packed on the host at staging time (transposed to
[in, out] layout and cast to bf16) — a one-time model-load transform.
Activations (hidden, encoder_outputs, tokens) are staged untouched.
"""
import sys

if "/opt/trn_rl_repo" not in sys.path:
    sys.path.insert(0, "/opt/trn_rl_repo")

import numpy as np
import ml_dtypes

import concourse.bass as bass
import concourse.tile as tile
from concourse import bacc, mybir
from concourse import bass_utils
from concourse.masks import make_identity

BF16 = mybir.dt.bfloat16
F32 = mybir.dt.float32
I32 = mybir.dt.int32
AF = mybir.ActivationFunctionType

H, V, B, L = 512, 50257, 256, 50
NC = 8
BS = B // NC            # 32 batch rows per core
VS = (V + NC - 1) // NC  # 6283 vocab columns per core
VPAD = VS * NC           # 50264
KT = H // 128            # 4 contraction tiles of 128
RG = [list(range(NC))]

# vocab column tiles (PSUM bank limit: 512 f32 per matmul)
N_TILES = []
_off = 0
while _off < VS:
    n = min(512, VS - _off)
    N_TILES.append((_off, n))
    _off += n

_CACHE = {}


def _build():
    nc = bacc.Bacc("TRN2", target_bir_lowering=False, debug=False, num_devices=NC)

    # ---- I/O ----
    tok = nc.dram_tensor("tok", [BS, 1], I32, kind="ExternalInput")
    h0 = nc.dram_tensor("h0", [BS, H], F32, kind="ExternalInput")
    enc = nc.dram_tensor("enc", [L, BS, H], F32, kind="ExternalInput")
    embt = nc.dram_tensor("embt", [V, H], BF16, kind="ExternalInput")
    attn_wT = nc.dram_tensor("attn_wT", [2 * H, L], BF16, kind="ExternalInput")
    attn_b = nc.dram_tensor("attn_b", [1, L], BF16, kind="ExternalInput")
    comb_wT = nc.dram_tensor("comb_wT", [2 * H, H], BF16, kind="ExternalInput")
    comb_b = nc.dram_tensor("comb_b", [1, H], BF16, kind="ExternalInput")
    w_ihT = nc.dram_tensor("w_ihT", [H, 3 * H], BF16, kind="ExternalInput")
    w_hhT = nc.dram_tensor("w_hhT", [H, 3 * H], BF16, kind="ExternalInput")
    b_ih = nc.dram_tensor("b_ih", [1, 3 * H], BF16, kind="ExternalInput")
    b_hh = nc.dram_tensor("b_hh", [1, 3 * H], BF16, kind="ExternalInput")
    fc_wT = nc.dram_tensor("fc_wT", [H, VS], BF16, kind="ExternalInput")
    fc_b = nc.dram_tensor("fc_b", [1, VS], BF16, kind="ExternalInput")

    out_logp = nc.dram_tensor("out_logp", [B, VS], F32, kind="ExternalOutput")
    out_h = nc.dram_tensor("out_h", [BS, H], F32, kind="ExternalOutput")
    out_attn = nc.dram_tensor("out_attn", [BS, L], F32, kind="ExternalOutput")

    cc_h_in = nc.dram_tensor("cc_h_in", [BS, H], BF16)
    cc_h_out = nc.dram_tensor("cc_h_out", [B, H], BF16, addr_space="Shared")
    cc_s_in = nc.dram_tensor("cc_s_in", [1, B], F32)
    cc_s_out = nc.dram_tensor("cc_s_out", [NC, B], F32, addr_space="Shared")
    cc_d_in = nc.dram_tensor("cc_d_in", [1, 8], F32)
    cc_d_out = nc.dram_tensor("cc_d_out", [NC, 8], F32, addr_space="Shared")

    with tile.TileContext(nc) as tc:
        with (
            tc.tile_pool(name="singles", bufs=1) as sg,
            tc.tile_pool(name="work", bufs=2) as wk,
            tc.tile_pool(name="encp", bufs=8) as encp,
            tc.tile_pool(name="outp", bufs=4) as outp,
            tc.tile_pool(name="trp", bufs=2, space="PSUM") as trp,
            tc.tile_pool(name="gp", bufs=3, space="PSUM") as gp,
            tc.tile_pool(name="zp", bufs=3, space="PSUM") as zp,
        ):
            # ---- dummy collective: absorbs the expensive first-collective
            # setup concurrently with the front instead of on the h path ----
            dmy = sg.tile([1, 8], F32, tag="dmy")
            nc.vector.memset(dmy[:], 0.0)
            nc.sync.dma_start(out=cc_d_in.ap(), in_=dmy[:])
            nc.gpsimd.collective_compute(
                "AllGather", mybir.AluOpType.bypass, replica_groups=RG,
                ins=[cc_d_in.ap()], outs=[cc_d_out.ap()],
            )

            # ---- constants ----
            id_bf = sg.tile([128, 128], BF16, tag="id_bf")
            make_identity(nc, id_bf[:])
            id_f = sg.tile([128, 128], F32, tag="id_f")
            make_identity(nc, id_f[:])
            ones_bf = sg.tile([1, 128], BF16, tag="ones")
            nc.vector.memset(ones_bf[:], 1.0)

            # ---- embedding gather ----
            tok_sb = sg.tile([BS, 1], I32, tag="tok")
            nc.sync.dma_start(out=tok_sb[:], in_=tok.ap())
            emb_own = sg.tile([BS, H], BF16, tag="embrow")
            nc.gpsimd.indirect_dma_start(
                out=emb_own[:], out_offset=None, in_=embt.ap(),
                in_offset=bass.IndirectOffsetOnAxis(ap=tok_sb[:, 0:1], axis=0),
            )
            h0_sb = sg.tile([BS, H], F32, tag="h0")
            nc.sync.dma_start(out=h0_sb[:], in_=h0.ap())

            # ---- feature-major transposes of embedded and h0 ----
            embT = []
            h0T = []
            for k in range(KT):
                pe = trp.tile([128, BS], BF16, tag="tr")
                nc.tensor.transpose(out=pe[:], in_=emb_own[:, 128 * k:128 * (k + 1)],
                                    identity=id_bf[:BS, :BS])
                t = sg.tile([128, BS], BF16, tag=f"embT{k}")
                nc.vector.tensor_copy(out=t[:], in_=pe[:])
                embT.append(t)

                pf = trp.tile([128, BS], F32, tag="tr")
                nc.tensor.transpose(out=pf[:], in_=h0_sb[:, 128 * k:128 * (k + 1)],
                                    identity=id_f[:BS, :BS])
                t2 = sg.tile([128, BS], BF16, tag=f"h0T{k}")
                nc.vector.tensor_copy(out=t2[:], in_=pf[:])
                h0T.append(t2)

            # ---- attention scores + softmax ----
            aw_sb = sg.tile([128, 2 * KT, L], BF16, tag="aw")
            nc.sync.dma_start(out=aw_sb[:],
                              in_=attn_wT.ap().rearrange("(k p) l -> p k l", p=128))
            ab_sb = sg.tile([1, L], BF16, tag="ab")
            nc.sync.dma_start(out=ab_sb[:], in_=attn_b.ap())

            psc = gp.tile([BS, 512], F32, tag="gpsum")
            for k in range(KT):
                nc.tensor.matmul(out=psc[:, :L], lhsT=embT[k][:], rhs=aw_sb[:, k, :],
                                 start=(k == 0), stop=False)
            for k in range(KT):
                nc.tensor.matmul(out=psc[:, :L], lhsT=h0T[k][:], rhs=aw_sb[:, KT + k, :],
                                 start=False, stop=False)
            nc.tensor.matmul(out=psc[:, :L], lhsT=ones_bf[0:1, :BS], rhs=ab_sb[:],
                             start=False, stop=True)

            # scores are tiny (|s| < ~1), exp is safe without max subtraction
            e_sb = sg.tile([BS, L], F32, tag="esb")
            ssum = sg.tile([BS, 1], F32, tag="ssum")
            nc.scalar.activation(out=e_sb[:], in_=psc[:, :L], func=AF.Exp,
                                 accum_out=ssum[:])
            rinv = sg.tile([BS, 1], F32, tag="rinv")
            nc.vector.reciprocal(rinv[:], ssum[:])
            attnw = sg.tile([BS, L], F32, tag="attnw")
            nc.vector.tensor_scalar_mul(attnw[:], e_sb[:], rinv[:, 0:1])
            nc.sync.dma_start(out=out_attn.ap(), in_=attnw[:])

            # ---- attn_applied via masked accumulating matmuls ----
            pwt = trp.tile([L, BS], F32, tag="tr")
            nc.tensor.transpose(out=pwt[:], in_=attnw[:], identity=id_f[:BS, :BS])
            wt_bf = sg.tile([L, BS], BF16, tag="wtbf")
            nc.vector.tensor_copy(out=wt_bf[:], in_=pwt[:])
            wmask = sg.tile([L, BS, BS], BF16, tag="wmask")
            nc.vector.tensor_copy(
                out=wmask[:],
                in_=wt_bf[:].rearrange("l (o j) -> l o j", o=1).to_broadcast([L, BS, BS]),
            )
            nc.gpsimd.affine_select(
                out=wmask[:], in_=wmask[:], compare_op=mybir.AluOpType.is_equal,
                fill=0.0, base=0, pattern=[[-1, BS], [1, BS]], channel_multiplier=0,
            )
            patt = gp.tile([BS, 512], F32, tag="gpsum")
            for b in range(BS):
                et = encp.tile([L, H], F32, tag="enc")
                nc.sync.dma_start(out=et[:], in_=enc.ap()[:, b, :])
                ebf = et[:].bitcast(BF16).rearrange("l (n two) -> l n two", two=2)[:, :, 1]
                nc.tensor.matmul(out=patt[:], lhsT=wmask[:, b, :], rhs=ebf,
                                 start=(b == 0), stop=(b == BS - 1))
            aa_bf = sg.tile([BS, H], BF16, tag="aabf")
            nc.vector.tensor_copy(out=aa_bf[:], in_=patt[:])

            aaT = []
            for k in range(KT):
                pe = trp.tile([128, BS], BF16, tag="tr")
                nc.tensor.transpose(out=pe[:], in_=aa_bf[:, 128 * k:128 * (k + 1)],
                                    identity=id_bf[:BS, :BS])
                t = sg.tile([128, BS], BF16, tag=f"aaT{k}")
                nc.vector.tensor_copy(out=t[:], in_=pe[:])
                aaT.append(t)

            # ---- combine + relu ----
            cw_sb = sg.tile([128, 2 * KT, H], BF16, tag="cw")
            nc.sync.dma_start(out=cw_sb[:],
                              in_=comb_wT.ap().rearrange("(k p) n -> p k n", p=128))
            cb_sb = sg.tile([1, H], BF16, tag="cb")
            nc.sync.dma_start(out=cb_sb[:], in_=comb_b.ap())
            px = gp.tile([BS, 512], F32, tag="gpsum")
            for k in range(KT):
                nc.tensor.matmul(out=px[:, :H], lhsT=embT[k][:], rhs=cw_sb[:, k, :],
                                 start=(k == 0), stop=False)
            for k in range(KT):
                nc.tensor.matmul(out=px[:, :H], lhsT=aaT[k][:], rhs=cw_sb[:, KT + k, :],
                                 start=False, stop=False)
            nc.tensor.matmul(out=px[:, :H], lhsT=ones_bf[0:1, :BS], rhs=cb_sb[:],
                             start=False, stop=True)
            x_bf = sg.tile([BS, H], BF16, tag="xbf")
            nc.scalar.activation(out=x_bf[:], in_=px[:, :H], func=AF.Relu)

            xT = []
            for k in range(KT):
                pe = trp.tile([128, BS], BF16, tag="tr")
                nc.tensor.transpose(out=pe[:], in_=x_bf[:, 128 * k:128 * (k + 1)],
                                    identity=id_bf[:BS, :BS])
                t = sg.tile([128, BS], BF16, tag=f"xT{k}")
                nc.vector.tensor_copy(out=t[:], in_=pe[:])
                xT.append(t)

            # ---- GRU cell ----
            wih_sb = sg.tile([128, KT, 3 * H], BF16, tag="wih")
            nc.sync.dma_start(out=wih_sb[:],
                              in_=w_ihT.ap().rearrange("(k p) n -> p k n", p=128))
            whh_sb = sg.tile([128, KT, 3 * H], BF16, tag="whh")
            nc.sync.dma_start(out=whh_sb[:],
                              in_=w_hhT.ap().rearrange("(k p) n -> p k n", p=128))
            bih_sb = sg.tile([1, 3 * H], BF16, tag="bih")
            nc.sync.dma_start(out=bih_sb[:], in_=b_ih.ap())
            bhh_sb = sg.tile([1, 3 * H], BF16, tag="bhh")
            nc.sync.dma_start(out=bhh_sb[:], in_=b_hh.ap())

            gi_sb = []
            for j in range(3):
                pg = gp.tile([BS, 512], F32, tag="gpsum")
                for k in range(KT):
                    nc.tensor.matmul(out=pg[:, :H], lhsT=xT[k][:],
                                     rhs=wih_sb[:, k, H * j:H * (j + 1)],
                                     start=(k == 0), stop=False)
                nc.tensor.matmul(out=pg[:, :H], lhsT=ones_bf[0:1, :BS],
                                 rhs=bih_sb[:, H * j:H * (j + 1)],
                                 start=False, stop=True)
                t = sg.tile([BS, H], F32, tag=f"gi{j}")
                nc.vector.tensor_copy(out=t[:], in_=pg[:, :H])
                gi_sb.append(t)

            r_sb = sg.tile([BS, H], F32, tag="r")
            z_gate = sg.tile([BS, H], F32, tag="zg")
            n_sb = sg.tile([BS, H], F32, tag="n")
            hnew = sg.tile([BS, H], F32, tag="hnew")
            for j in range(3):
                pg = gp.tile([BS, 512], F32, tag="gpsum")
                for k in range(KT):
                    nc.tensor.matmul(out=pg[:, :H], lhsT=h0T[k][:],
                                     rhs=whh_sb[:, k, H * j:H * (j + 1)],
                                     start=(k == 0), stop=False)
                nc.tensor.matmul(out=pg[:, :H], lhsT=ones_bf[0:1, :BS],
                                 rhs=bhh_sb[:, H * j:H * (j + 1)],
                                 start=False, stop=True)
                if j == 0:
                    pre = sg.tile([BS, H], F32, tag="pre0")
                    nc.vector.tensor_add(out=pre[:], in0=gi_sb[0][:], in1=pg[:, :H])
                    nc.scalar.activation(out=r_sb[:], in_=pre[:], func=AF.Sigmoid)
                elif j == 1:
                    pre = sg.tile([BS, H], F32, tag="pre1")
                    nc.vector.tensor_add(out=pre[:], in0=gi_sb[1][:], in1=pg[:, :H])
                    nc.scalar.activation(out=z_gate[:], in_=pre[:], func=AF.Sigmoid)
                else:
                    hnr = sg.tile([BS, H], F32, tag="hnr")
                    nc.vector.tensor_mul(out=hnr[:], in0=r_sb[:], in1=pg[:, :H])
                    pre = sg.tile([BS, H], F32, tag="pre2")
                    nc.vector.tensor_add(out=pre[:], in0=gi_sb[2][:], in1=hnr[:])
                    nc.scalar.activation(out=n_sb[:], in_=pre[:], func=AF.Tanh)

            d_sb = sg.tile([BS, H], F32, tag="d")
            nc.vector.tensor_tensor(out=d_sb[:], in0=h0_sb[:], in1=n_sb[:],
                                    op=mybir.AluOpType.subtract)
            e2_sb = sg.tile([BS, H], F32, tag="e2")
            nc.vector.tensor_mul(out=e2_sb[:], in0=z_gate[:], in1=d_sb[:])
            nc.vector.tensor_add(out=hnew[:], in0=n_sb[:], in1=e2_sb[:])
            nc.sync.dma_start(out=out_h.ap(), in_=hnew[:])

            # ---- all-gather h_new (bf16) ----
            h_bf = sg.tile([BS, H], BF16, tag="hbf")
            nc.scalar.activation(out=h_bf[:], in_=hnew[:], func=AF.Copy)
            nc.sync.dma_start(out=cc_h_in.ap(), in_=h_bf[:])
            nc.gpsimd.collective_compute(
                "AllGather", mybir.AluOpType.bypass, replica_groups=RG,
                ins=[cc_h_in.ap()], outs=[cc_h_out.ap()],
            )

            hT = [sg.tile([128, B], BF16, tag=f"hT{k}", name=f"hT{k}") for k in range(KT)]
            for bt in range(2):
                hf = wk.tile([128, H], BF16, tag="hfull")
                nc.sync.dma_start(out=hf[:], in_=cc_h_out.ap()[128 * bt:128 * (bt + 1), :])
                for k in range(KT):
                    pe = trp.tile([128, 128], BF16, tag="tr")
                    nc.tensor.transpose(out=pe[:], in_=hf[:, 128 * k:128 * (k + 1)],
                                        identity=id_bf[:])
                    nc.vector.tensor_copy(out=hT[k][:, 128 * bt:128 * (bt + 1)], in_=pe[:])

            # ---- fc matmul + exp/σ stats ----
            # fc_wT is made fully SBUF-resident via 4 big DMAs that carry no
            # dependency on the front, so they stream during front + AllGather.
            fcb_sb = sg.tile([1, VS], BF16, tag="fcb")
            nc.sync.dma_start(out=fcb_sb[:], in_=fc_b.ap())
            wz_res = sg.tile([128, KT, VS], BF16, tag="wzres")
            for k in range(KT):
                nc.sync.dma_start(out=wz_res[:, k, :],
                                  in_=fc_wT.ap()[128 * k:128 * (k + 1), :])
            z_sb = [sg.tile([128, VS], BF16, tag=f"z{bt}", name=f"z{bt}") for bt in range(2)]
            stats = [sg.tile([128, len(N_TILES)], F32, tag=f"st{bt}", name=f"stats{bt}") for bt in range(2)]

            for nt, (ncur, n) in enumerate(N_TILES):
                for bt in range(2):
                    pz = zp.tile([128, 512], F32, tag="zpsum")
                    for k in range(KT):
                        nc.tensor.matmul(out=pz[:, :n],
                                         lhsT=hT[k][:, 128 * bt:128 * (bt + 1)],
                                         rhs=wz_res[:, k, ncur:ncur + n],
                                         start=(k == 0), stop=False)
                    nc.tensor.matmul(out=pz[:, :n], lhsT=ones_bf[0:1, :],
                                     rhs=fcb_sb[:, ncur:ncur + n], start=False, stop=True)
                    nc.vector.tensor_copy(out=z_sb[bt][:, ncur:ncur + n], in_=pz[:, :n])
                    esc = wk.tile([128, 512], BF16, tag="esc")
                    nc.scalar.activation(out=esc[:, :n], in_=pz[:, :n],
                                         func=AF.Exp, accum_out=stats[bt][:, nt:nt + 1])

            # ---- all-gather softmax denominators ----
            for bt in range(2):
                s_own = sg.tile([128, 1], F32, tag=f"sown{bt}")
                nc.vector.reduce_sum(s_own[:], stats[bt][:, 0:len(N_TILES)],
                                     axis=mybir.AxisListType.X)
                pt = trp.tile([1, 128], F32, tag="tr")
                nc.tensor.transpose(out=pt[:], in_=s_own[:], identity=id_f[:])
                srow = sg.tile([1, 128], F32, tag=f"srow{bt}")
                nc.vector.tensor_copy(out=srow[:], in_=pt[:])
                nc.sync.dma_start(out=cc_s_in.ap()[:, 128 * bt:128 * (bt + 1)], in_=srow[:])
            nc.gpsimd.collective_compute(
                "AllGather", mybir.AluOpType.bypass, replica_groups=RG,
                ins=[cc_s_in.ap()], outs=[cc_s_out.ap()],
            )

            lse = []
            for bt in range(2):
                s_all = sg.tile([128, NC], F32, tag=f"sall{bt}")
                nc.sync.dma_start(
                    out=s_all[:],
                    in_=cc_s_out.ap().rearrange("r b -> b r")[128 * bt:128 * (bt + 1), :])
                s_tot = sg.tile([128, 1], F32, tag=f"stot{bt}")
                nc.vector.reduce_sum(s_tot[:], s_all[:], axis=mybir.AxisListType.X)
                ls = sg.tile([128, 1], F32, tag=f"lse{bt}")
                nc.scalar.activation(out=ls[:], in_=s_tot[:], func=AF.Ln)
                lse.append(ls)

            # ---- normalize + store ----
            for bt in range(2):
                for ncur, n in N_TILES:
                    o_t = outp.tile([128, 512], F32, tag="ost")
                    nc.vector.tensor_scalar_sub(o_t[:, :n], z_sb[bt][:, ncur:ncur + n],
                                                lse[bt][:, 0:1])
                    nc.sync.dma_start(
                        out=out_logp.ap()[128 * bt:128 * (bt + 1), ncur:ncur + n],
                        in_=o_t[:, :n])

    nc.compile()
    return nc


def _stage(inputs):
    """Build the 8 per-core in_maps from the full-size inputs."""
    bf = ml_dtypes.bfloat16
    tok = np.asarray(inputs["input_tokens"]).astype(np.int32).reshape(B, 1)
    hidden = np.ascontiguousarray(np.asarray(inputs["hidden"], np.float32))[0]  # [B,H]
    enc = np.ascontiguousarray(np.asarray(inputs["encoder_outputs"], np.float32))
    emb_bf = np.asarray(inputs["emb"], np.float32).astype(bf)
    attn_wT = np.ascontiguousarray(np.asarray(inputs["attn_w"], np.float32).T).astype(bf)
    attn_b = np.asarray(inputs["attn_b"], np.float32).reshape(1, L).astype(bf)
    comb_wT = np.ascontiguousarray(np.asarray(inputs["comb_w"], np.float32).T).astype(bf)
    comb_b = np.asarray(inputs["comb_b"], np.float32).reshape(1, H).astype(bf)
    w_ihT = np.ascontiguousarray(np.asarray(inputs["w_ih"], np.float32).T).astype(bf)
    w_hhT = np.ascontiguousarray(np.asarray(inputs["w_hh"], np.float32).T).astype(bf)
    b_ih = np.asarray(inputs["b_ih"], np.float32).reshape(1, 3 * H).astype(bf)
    b_hh = np.asarray(inputs["b_hh"], np.float32).reshape(1, 3 * H).astype(bf)

    fc_w = np.asarray(inputs["fc_w"], np.float32)
    fc_b = np.asarray(inputs["fc_b"], np.float32)
    fc_w_pad = np.zeros((VPAD, H), np.float32)
    fc_w_pad[:V] = fc_w
    fc_b_pad = np.full((VPAD,), -1e30, np.float32)
    fc_b_pad[:V] = fc_b

    in_maps = []
    for c in range(NC):
        b0 = c * BS
        v0 = c * VS
        in_maps.append({
            "tok": tok[b0:b0 + BS],
            "h0": np.ascontiguousarray(hidden[b0:b0 + BS]),
            "enc": np.ascontiguousarray(enc[:, b0:b0 + BS, :]),
            "embt": emb_bf,
            "attn_wT": attn_wT,
            "attn_b": attn_b,
            "comb_wT": comb_wT,
            "comb_b": comb_b,
            "w_ihT": w_ihT,
            "w_hhT": w_hhT,
            "b_ih": b_ih,
            "b_hh": b_hh,
            "fc_wT": np.ascontiguousarray(fc_w_pad[v0:v0 + VS].T).astype(bf),
            "fc_b": fc_b_pad[v0:v0 + VS].reshape(1, VS).astype(bf),
        })
    return in_maps


def _run(inputs, trace=False, trace_cores=None):
    if "nc" not in _CACHE:
        _CACHE["nc"] = _build()
    nc = _CACHE["nc"]
    in_maps = _stage(inputs)
    res = bass_utils.run_bass_kernel_spmd(
        nc, in_maps, core_ids=list(range(NC)), trace=trace, trace_cores=trace_cores)
    logp = np.concatenate([res.results[c]["out_logp"] for c in range(NC)], axis=1)[:, :V]
    h_new = np.concatenate([res.results[c]["out_h"] for c in range(NC)], axis=0)[None]
    attnw = np.concatenate([res.results[c]["out_attn"] for c in range(NC)], axis=0)
    return (logp, h_new, attnw), res


def kernel(**inputs):
    out, _ = _run(inputs, trace=False)
    return out


# revision 14
# speedup vs baseline: 1.2133x; 1.1544x over previous
"""AttnDecoderRNN single-step decoder on 8 Trainium2 NeuronCores.

Sharding:
  - Front (embedding gather, Bahdanau attention, combine+relu, GRU cell):
    data-parallel over batch (32 rows/core).
  - h_new all-gathered (bf16) across the 8 cores.
  - Final fc + log_softmax: tensor-parallel over the vocab dimension
    (6283 columns/core); log-softmax denominators all-gathered and the
    normalization applied locally.

Weights are pre-packed on the host at staging time (transposed to
[in, out] layout and cast to bf16) — a one-time model-load transform.
Activations (hidden, encoder_outputs, tokens) are staged untouched.
"""
import sys

if "/opt/trn_rl_repo" not in sys.path:
    sys.path.insert(0, "/opt/trn_rl_repo")

import numpy as np
import ml_dtypes

import concourse.bass as bass
import concourse.tile as tile
from concourse import bacc, mybir
from concourse import bass_utils
from concourse.masks import make_identity

BF16 = mybir.dt.bfloat16
F32 = mybir.dt.float32
I32 = mybir.dt.int32
AF = mybir.ActivationFunctionType

H, V, B, L = 512, 50257, 256, 50
NC = 8
BS = B // NC            # 32 batch rows per core
VS = (V + NC - 1) // NC  # 6283 vocab columns per core
VPAD = VS * NC           # 50264
KT = H // 128            # 4 contraction tiles of 128
RG = [list(range(NC))]

# vocab column tiles (PSUM bank limit: 512 f32 per matmul)
N_TILES = []
_off = 0
while _off < VS:
    n = min(512, VS - _off)
    N_TILES.append((_off, n))
    _off += n

_CACHE = {}


def _build():
    nc = bacc.Bacc("TRN2", target_bir_lowering=False, debug=False, num_devices=NC)

    # ---- I/O ----
    tok = nc.dram_tensor("tok", [BS, 1], I32, kind="ExternalInput")
    h0 = nc.dram_tensor("h0", [BS, H], F32, kind="ExternalInput")
    enc = nc.dram_tensor("enc", [L, BS, H], F32, kind="ExternalInput")
    embt = nc.dram_tensor("embt", [V, H], BF16, kind="ExternalInput")
    attn_wT = nc.dram_tensor("attn_wT", [2 * H, L], BF16, kind="ExternalInput")
    attn_b = nc.dram_tensor("attn_b", [1, L], BF16, kind="ExternalInput")
    comb_wT = nc.dram_tensor("comb_wT", [2 * H, H], BF16, kind="ExternalInput")
    comb_b = nc.dram_tensor("comb_b", [1, H], BF16, kind="ExternalInput")
    w_ihT = nc.dram_tensor("w_ihT", [H, 3 * H], BF16, kind="ExternalInput")
    w_hhT = nc.dram_tensor("w_hhT", [H, 3 * H], BF16, kind="ExternalInput")
    b_ih = nc.dram_tensor("b_ih", [1, 3 * H], BF16, kind="ExternalInput")
    b_hh = nc.dram_tensor("b_hh", [1, 3 * H], BF16, kind="ExternalInput")
    fc_wT = nc.dram_tensor("fc_wT", [H, VS], BF16, kind="ExternalInput")
    fc_b = nc.dram_tensor("fc_b", [1, VS], BF16, kind="ExternalInput")

    out_logp = nc.dram_tensor("out_logp", [B, VS], F32, kind="ExternalOutput")
    out_h = nc.dram_tensor("out_h", [BS, H], F32, kind="ExternalOutput")
    out_attn = nc.dram_tensor("out_attn", [BS, L], F32, kind="ExternalOutput")

    cc_h_in = nc.dram_tensor("cc_h_in", [BS, H], BF16)
    cc_h_out = nc.dram_tensor("cc_h_out", [B, H], BF16, addr_space="Shared")
    cc_s_in = [nc.dram_tensor(f"cc_s_in{bt}", [1, 128], F32) for bt in range(2)]
    cc_s_out = [nc.dram_tensor(f"cc_s_out{bt}", [NC, 128], F32, addr_space="Shared")
                for bt in range(2)]
    cc_d_in = nc.dram_tensor("cc_d_in", [1, 8], F32)
    cc_d_out = nc.dram_tensor("cc_d_out", [NC, 8], F32, addr_space="Shared")

    with tile.TileContext(nc) as tc:
        with (
            tc.tile_pool(name="singles", bufs=1) as sg,
            tc.tile_pool(name="work", bufs=2) as wk,
            tc.tile_pool(name="encp", bufs=8) as encp,
            tc.tile_pool(name="outp", bufs=4) as outp,
            tc.tile_pool(name="trp", bufs=2, space="PSUM") as trp,
            tc.tile_pool(name="gp", bufs=3, space="PSUM") as gp,
            tc.tile_pool(name="zp", bufs=3, space="PSUM") as zp,
        ):
            # ---- dummy collective: absorbs the expensive first-collective
            # setup concurrently with the front instead of on the h path ----
            dmy = sg.tile([1, 8], F32, tag="dmy")
            nc.vector.memset(dmy[:], 0.0)
            nc.sync.dma_start(out=cc_d_in.ap(), in_=dmy[:])
            nc.gpsimd.collective_compute(
                "AllGather", mybir.AluOpType.bypass, replica_groups=RG,
                ins=[cc_d_in.ap()], outs=[cc_d_out.ap()],
            )

            # ---- constants ----
            id_bf = sg.tile([128, 128], BF16, tag="id_bf")
            make_identity(nc, id_bf[:])
            id_f = sg.tile([128, 128], F32, tag="id_f")
            make_identity(nc, id_f[:])
            ones_bf = sg.tile([1, 128], BF16, tag="ones")
            nc.vector.memset(ones_bf[:], 1.0)

            # ---- embedding gather ----
            tok_sb = sg.tile([BS, 1], I32, tag="tok")
            nc.sync.dma_start(out=tok_sb[:], in_=tok.ap())
            emb_own = sg.tile([BS, H], BF16, tag="embrow")
            nc.gpsimd.indirect_dma_start(
                out=emb_own[:], out_offset=None, in_=embt.ap(),
                in_offset=bass.IndirectOffsetOnAxis(ap=tok_sb[:, 0:1], axis=0),
            )
            h0_sb = sg.tile([BS, H], F32, tag="h0")
            nc.sync.dma_start(out=h0_sb[:], in_=h0.ap())

            # ---- feature-major transposes of embedded and h0 ----
            embT = []
            h0T = []
            for k in range(KT):
                pe = trp.tile([128, BS], BF16, tag="tr")
                nc.tensor.transpose(out=pe[:], in_=emb_own[:, 128 * k:128 * (k + 1)],
                                    identity=id_bf[:BS, :BS])
                t = sg.tile([128, BS], BF16, tag=f"embT{k}")
                nc.vector.tensor_copy(out=t[:], in_=pe[:])
                embT.append(t)

                pf = trp.tile([128, BS], F32, tag="tr")
                nc.tensor.transpose(out=pf[:], in_=h0_sb[:, 128 * k:128 * (k + 1)],
                                    identity=id_f[:BS, :BS])
                t2 = sg.tile([128, BS], BF16, tag=f"h0T{k}")
                nc.vector.tensor_copy(out=t2[:], in_=pf[:])
                h0T.append(t2)

            # ---- attention scores + softmax ----
            aw_sb = sg.tile([128, 2 * KT, L], BF16, tag="aw")
            nc.sync.dma_start(out=aw_sb[:],
                              in_=attn_wT.ap().rearrange("(k p) l -> p k l", p=128))
            ab_sb = sg.tile([1, L], BF16, tag="ab")
            nc.sync.dma_start(out=ab_sb[:], in_=attn_b.ap())

            psc = gp.tile([BS, 512], F32, tag="gpsum")
            for k in range(KT):
                nc.tensor.matmul(out=psc[:, :L], lhsT=embT[k][:], rhs=aw_sb[:, k, :],
                                 start=(k == 0), stop=False)
            for k in range(KT):
                nc.tensor.matmul(out=psc[:, :L], lhsT=h0T[k][:], rhs=aw_sb[:, KT + k, :],
                                 start=False, stop=False)
            nc.tensor.matmul(out=psc[:, :L], lhsT=ones_bf[0:1, :BS], rhs=ab_sb[:],
                             start=False, stop=True)

            # scores are tiny (|s| < ~1), exp is safe without max subtraction
            e_sb = sg.tile([BS, L], F32, tag="esb")
            ssum = sg.tile([BS, 1], F32, tag="ssum")
            nc.scalar.activation(out=e_sb[:], in_=psc[:, :L], func=AF.Exp,
                                 accum_out=ssum[:])
            rinv = sg.tile([BS, 1], F32, tag="rinv")
            nc.vector.reciprocal(rinv[:], ssum[:])
            # attn_weights output is produced off the critical path; the
            # einsum uses unnormalized exp scores and rescales its output.
            attnw = sg.tile([BS, L], F32, tag="attnw")
            nc.vector.tensor_scalar_mul(attnw[:], e_sb[:], rinv[:, 0:1])
            nc.sync.dma_start(out=out_attn.ap(), in_=attnw[:])

            # ---- attn_applied via masked accumulating matmuls ----
            pwt = trp.tile([L, BS], F32, tag="tr")
            nc.tensor.transpose(out=pwt[:], in_=e_sb[:], identity=id_f[:BS, :BS])
            wt_bf = sg.tile([L, BS], BF16, tag="wtbf")
            nc.vector.tensor_copy(out=wt_bf[:], in_=pwt[:])
            wmask = sg.tile([L, BS, BS], BF16, tag="wmask")
            nc.vector.tensor_copy(
                out=wmask[:],
                in_=wt_bf[:].rearrange("l (o j) -> l o j", o=1).to_broadcast([L, BS, BS]),
            )
            nc.gpsimd.affine_select(
                out=wmask[:], in_=wmask[:], compare_op=mybir.AluOpType.is_equal,
                fill=0.0, base=0, pattern=[[-1, BS], [1, BS]], channel_multiplier=0,
            )
            patt = gp.tile([BS, 512], F32, tag="gpsum")
            for b in range(BS):
                et = encp.tile([L, H], F32, tag="enc")
                nc.sync.dma_start(out=et[:], in_=enc.ap()[:, b, :])
                ebf = et[:].bitcast(BF16).rearrange("l (n two) -> l n two", two=2)[:, :, 1]
                nc.tensor.matmul(out=patt[:], lhsT=wmask[:, b, :], rhs=ebf,
                                 start=(b == 0), stop=(b == BS - 1))
            aa_bf = sg.tile([BS, H], BF16, tag="aabf")
            nc.vector.tensor_scalar_mul(aa_bf[:], patt[:], rinv[:, 0:1])

            aaT = []
            for k in range(KT):
                pe = trp.tile([128, BS], BF16, tag="tr")
                nc.tensor.transpose(out=pe[:], in_=aa_bf[:, 128 * k:128 * (k + 1)],
                                    identity=id_bf[:BS, :BS])
                t = sg.tile([128, BS], BF16, tag=f"aaT{k}")
                nc.vector.tensor_copy(out=t[:], in_=pe[:])
                aaT.append(t)

            # ---- combine + relu ----
            cw_sb = sg.tile([128, 2 * KT, H], BF16, tag="cw")
            nc.sync.dma_start(out=cw_sb[:],
                              in_=comb_wT.ap().rearrange("(k p) n -> p k n", p=128))
            cb_sb = sg.tile([1, H], BF16, tag="cb")
            nc.sync.dma_start(out=cb_sb[:], in_=comb_b.ap())
            px = gp.tile([BS, 512], F32, tag="gpsum")
            for k in range(KT):
                nc.tensor.matmul(out=px[:, :H], lhsT=embT[k][:], rhs=cw_sb[:, k, :],
                                 start=(k == 0), stop=False)
            for k in range(KT):
                nc.tensor.matmul(out=px[:, :H], lhsT=aaT[k][:], rhs=cw_sb[:, KT + k, :],
                                 start=False, stop=False)
            nc.tensor.matmul(out=px[:, :H], lhsT=ones_bf[0:1, :BS], rhs=cb_sb[:],
                             start=False, stop=True)
            x_bf = sg.tile([BS, H], BF16, tag="xbf")
            nc.scalar.activation(out=x_bf[:], in_=px[:, :H], func=AF.Relu)

            xT = []
            for k in range(KT):
                pe = trp.tile([128, BS], BF16, tag="tr")
                nc.tensor.transpose(out=pe[:], in_=x_bf[:, 128 * k:128 * (k + 1)],
                                    identity=id_bf[:BS, :BS])
                t = sg.tile([128, BS], BF16, tag=f"xT{k}")
                nc.vector.tensor_copy(out=t[:], in_=pe[:])
                xT.append(t)

            # ---- GRU cell ----
            wih_sb = sg.tile([128, KT, 3 * H], BF16, tag="wih")
            nc.sync.dma_start(out=wih_sb[:],
                              in_=w_ihT.ap().rearrange("(k p) n -> p k n", p=128))
            whh_sb = sg.tile([128, KT, 3 * H], BF16, tag="whh")
            nc.sync.dma_start(out=whh_sb[:],
                              in_=w_hhT.ap().rearrange("(k p) n -> p k n", p=128))
            bih_sb = sg.tile([1, 3 * H], BF16, tag="bih")
            nc.sync.dma_start(out=bih_sb[:], in_=b_ih.ap())
            bhh_sb = sg.tile([1, 3 * H], BF16, tag="bhh")
            nc.sync.dma_start(out=bhh_sb[:], in_=b_hh.ap())

            gi_sb = []
            for j in range(3):
                pg = gp.tile([BS, 512], F32, tag="gpsum")
                for k in range(KT):
                    nc.tensor.matmul(out=pg[:, :H], lhsT=xT[k][:],
                                     rhs=wih_sb[:, k, H * j:H * (j + 1)],
                                     start=(k == 0), stop=False)
                nc.tensor.matmul(out=pg[:, :H], lhsT=ones_bf[0:1, :BS],
                                 rhs=bih_sb[:, H * j:H * (j + 1)],
                                 start=False, stop=True)
                t = sg.tile([BS, H], F32, tag=f"gi{j}")
                nc.vector.tensor_copy(out=t[:], in_=pg[:, :H])
                gi_sb.append(t)

            r_sb = sg.tile([BS, H], F32, tag="r")
            z_gate = sg.tile([BS, H], F32, tag="zg")
            n_sb = sg.tile([BS, H], F32, tag="n")
            hnew = sg.tile([BS, H], F32, tag="hnew")
            for j in range(3):
                pg = gp.tile([BS, 512], F32, tag="gpsum")
                for k in range(KT):
                    nc.tensor.matmul(out=pg[:, :H], lhsT=h0T[k][:],
                                     rhs=whh_sb[:, k, H * j:H * (j + 1)],
                                     start=(k == 0), stop=False)
                nc.tensor.matmul(out=pg[:, :H], lhsT=ones_bf[0:1, :BS],
                                 rhs=bhh_sb[:, H * j:H * (j + 1)],
                                 start=False, stop=True)
                if j == 0:
                    pre = sg.tile([BS, H], F32, tag="pre0")
                    nc.vector.tensor_add(out=pre[:], in0=gi_sb[0][:], in1=pg[:, :H])
                    nc.scalar.activation(out=r_sb[:], in_=pre[:], func=AF.Sigmoid)
                elif j == 1:
                    pre = sg.tile([BS, H], F32, tag="pre1")
                    nc.vector.tensor_add(out=pre[:], in0=gi_sb[1][:], in1=pg[:, :H])
                    nc.scalar.activation(out=z_gate[:], in_=pre[:], func=AF.Sigmoid)
                else:
                    hnr = sg.tile([BS, H], F32, tag="hnr")
                    nc.vector.tensor_mul(out=hnr[:], in0=r_sb[:], in1=pg[:, :H])
                    pre = sg.tile([BS, H], F32, tag="pre2")
                    nc.vector.tensor_add(out=pre[:], in0=gi_sb[2][:], in1=hnr[:])
                    nc.scalar.activation(out=n_sb[:], in_=pre[:], func=AF.Tanh)

            d_sb = sg.tile([BS, H], F32, tag="d")
            nc.vector.tensor_tensor(out=d_sb[:], in0=h0_sb[:], in1=n_sb[:],
                                    op=mybir.AluOpType.subtract)
            e2_sb = sg.tile([BS, H], F32, tag="e2")
            nc.vector.tensor_mul(out=e2_sb[:], in0=z_gate[:], in1=d_sb[:])
            # final add writes bf16 directly so the AllGather can fire without
            # an extra cast on the critical path; the f32 h_new output is
            # reconstructed from it off-path
            h_bf = sg.tile([BS, H], BF16, tag="hbf")
            nc.vector.tensor_add(out=h_bf[:], in0=n_sb[:], in1=e2_sb[:])
            nc.sync.dma_start(out=cc_h_in.ap(), in_=h_bf[:])
            nc.scalar.activation(out=hnew[:], in_=h_bf[:], func=AF.Copy)
            nc.sync.dma_start(out=out_h.ap(), in_=hnew[:])
            nc.gpsimd.collective_compute(
                "AllGather", mybir.AluOpType.bypass, replica_groups=RG,
                ins=[cc_h_in.ap()], outs=[cc_h_out.ap()],
            )

            hT = [sg.tile([128, B], BF16, tag=f"hT{k}", name=f"hT{k}") for k in range(KT)]
            for bt in range(2):
                hf = wk.tile([128, H], BF16, tag="hfull")
                nc.sync.dma_start(out=hf[:], in_=cc_h_out.ap()[128 * bt:128 * (bt + 1), :])
                for k in range(KT):
                    pe = trp.tile([128, 128], BF16, tag="tr")
                    nc.tensor.transpose(out=pe[:], in_=hf[:, 128 * k:128 * (k + 1)],
                                        identity=id_bf[:])
                    nc.vector.tensor_copy(out=hT[k][:, 128 * bt:128 * (bt + 1)], in_=pe[:])

            # ---- fc matmul + exp/σ stats ----
            # fc_wT is made fully SBUF-resident via 4 big DMAs that carry no
            # dependency on the front, so they stream during front + AllGather.
            fcb_sb = sg.tile([1, VS], BF16, tag="fcb")
            nc.sync.dma_start(out=fcb_sb[:], in_=fc_b.ap())
            wz_res = sg.tile([128, KT, VS], BF16, tag="wzres")
            for k in range(KT):
                nc.sync.dma_start(out=wz_res[:, k, :],
                                  in_=fc_wT.ap()[128 * k:128 * (k + 1), :])
            z_sb = [sg.tile([128, VS], BF16, tag=f"z{bt}", name=f"z{bt}") for bt in range(2)]
            stats = [sg.tile([128, len(N_TILES)], F32, tag=f"st{bt}", name=f"stats{bt}") for bt in range(2)]

            # bt-outer: batch-tile 0 finishes its matmuls, fires its stats
            # AllGather, and normalizes+stores while batch-tile 1's matmuls
            # are still running on the PE.
            for bt in range(2):
                for nt, (ncur, n) in enumerate(N_TILES):
                    pz = zp.tile([128, 512], F32, tag="zpsum")
                    for k in range(KT):
                        nc.tensor.matmul(out=pz[:, :n],
                                         lhsT=hT[k][:, 128 * bt:128 * (bt + 1)],
                                         rhs=wz_res[:, k, ncur:ncur + n],
                                         start=(k == 0), stop=False)
                    nc.tensor.matmul(out=pz[:, :n], lhsT=ones_bf[0:1, :],
                                     rhs=fcb_sb[:, ncur:ncur + n], start=False, stop=True)
                    nc.vector.tensor_copy(out=z_sb[bt][:, ncur:ncur + n], in_=pz[:, :n])
                    esc = wk.tile([128, 512], BF16, tag="esc")
                    nc.scalar.activation(out=esc[:, :n], in_=pz[:, :n],
                                         func=AF.Exp, accum_out=stats[bt][:, nt:nt + 1])

                # local softmax denominator for this batch tile -> all-gather
                s_own = sg.tile([128, 1], F32, tag=f"sown{bt}", name=f"sown{bt}")
                nc.vector.reduce_sum(s_own[:], stats[bt][:, 0:len(N_TILES)],
                                     axis=mybir.AxisListType.X)
                pt = trp.tile([1, 128], F32, tag="tr")
                nc.tensor.transpose(out=pt[:], in_=s_own[:], identity=id_f[:])
                srow = sg.tile([1, 128], F32, tag=f"srow{bt}", name=f"srow{bt}")
                nc.vector.tensor_copy(out=srow[:], in_=pt[:])
                nc.sync.dma_start(out=cc_s_in[bt].ap(), in_=srow[:])
                nc.gpsimd.collective_compute(
                    "AllGather", mybir.AluOpType.bypass, replica_groups=RG,
                    ins=[cc_s_in[bt].ap()], outs=[cc_s_out[bt].ap()],
                )
                s_all = sg.tile([128, NC], F32, tag=f"sall{bt}", name=f"sall{bt}")
                nc.sync.dma_start(out=s_all[:],
                                  in_=cc_s_out[bt].ap().rearrange("r b -> b r"))
                s_tot = sg.tile([128, 1], F32, tag=f"stot{bt}", name=f"stot{bt}")
                nc.vector.reduce_sum(s_tot[:], s_all[:], axis=mybir.AxisListType.X)
                ls = sg.tile([128, 1], F32, tag=f"lse{bt}", name=f"lse{bt}")
                nc.scalar.activation(out=ls[:], in_=s_tot[:], func=AF.Ln)

                # normalize + store this batch tile
                for ncur, n in N_TILES:
                    o_t = outp.tile([128, 512], F32, tag="ost")
                    nc.vector.tensor_scalar_sub(o_t[:, :n], z_sb[bt][:, ncur:ncur + n],
                                                ls[:, 0:1])
                    nc.sync.dma_start(
                        out=out_logp.ap()[128 * bt:128 * (bt + 1), ncur:ncur + n],
                        in_=o_t[:, :n])

    nc.compile()
    return nc


def _stage(inputs):
    """Build the 8 per-core in_maps from the full-size inputs."""
    bf = ml_dtypes.bfloat16
    tok = np.asarray(inputs["input_tokens"]).astype(np.int32).reshape(B, 1)
    hidden = np.ascontiguousarray(np.asarray(inputs["hidden"], np.float32))[0]  # [B,H]
    enc = np.ascontiguousarray(np.asarray(inputs["encoder_outputs"], np.float32))
    emb_bf = np.asarray(inputs["emb"], np.float32).astype(bf)
    attn_wT = np.ascontiguousarray(np.asarray(inputs["attn_w"], np.float32).T).astype(bf)
    attn_b = np.asarray(inputs["attn_b"], np.float32).reshape(1, L).astype(bf)
    comb_wT = np.ascontiguousarray(np.asarray(inputs["comb_w"], np.float32).T).astype(bf)
    comb_b = np.asarray(inputs["comb_b"], np.float32).reshape(1, H).astype(bf)
    w_ihT = np.ascontiguousarray(np.asarray(inputs["w_ih"], np.float32).T).astype(bf)
    w_hhT = np.ascontiguousarray(np.asarray(inputs["w_hh"], np.float32).T).astype(bf)
    b_ih = np.asarray(inputs["b_ih"], np.float32).reshape(1, 3 * H).astype(bf)
    b_hh = np.asarray(inputs["b_hh"], np.float32).reshape(1, 3 * H).astype(bf)

    fc_w = np.asarray(inputs["fc_w"], np.float32)
    fc_b = np.asarray(inputs["fc_b"], np.float32)
    fc_w_pad = np.zeros((VPAD, H), np.float32)
    fc_w_pad[:V] = fc_w
    fc_b_pad = np.full((VPAD,), -1e30, np.float32)
    fc_b_pad[:V] = fc_b

    in_maps = []
    for c in range(NC):
        b0 = c * BS
        v0 = c * VS
        in_maps.append({
            "tok": tok[b0:b0 + BS],
            "h0": np.ascontiguousarray(hidden[b0:b0 + BS]),
            "enc": np.ascontiguousarray(enc[:, b0:b0 + BS, :]),
            "embt": emb_bf,
            "attn_wT": attn_wT,
            "attn_b": attn_b,
            "comb_wT": comb_wT,
            "comb_b": comb_b,
            "w_ihT": w_ihT,
            "w_hhT": w_hhT,
            "b_ih": b_ih,
            "b_hh": b_hh,
            "fc_wT": np.ascontiguousarray(fc_w_pad[v0:v0 + VS].T).astype(bf),
            "fc_b": fc_b_pad[v0:v0 + VS].reshape(1, VS).astype(bf),
        })
    return in_maps


def _run(inputs, trace=False, trace_cores=None):
    if "nc" not in _CACHE:
        _CACHE["nc"] = _build()
    nc = _CACHE["nc"]
    in_maps = _stage(inputs)
    res = bass_utils.run_bass_kernel_spmd(
        nc, in_maps, core_ids=list(range(NC)), trace=trace, trace_cores=trace_cores)
    logp = np.concatenate([res.results[c]["out_logp"] for c in range(NC)], axis=1)[:, :V]
    h_new = np.concatenate([res.results[c]["out_h"] for c in range(NC)], axis=0)[None]
    attnw = np.concatenate([res.results[c]["out_attn"] for c in range(NC)], axis=0)
    return (logp, h_new, attnw), res


def kernel(**inputs):
    out, _ = _run(inputs, trace=False)
    return out
